# revision 1
# baseline (speedup 1.0000x reference)
"""RBF (Gaussian) kernel matrix on 8 TRN2 NeuronCores.

out[i, j] = exp(-gamma * ||x_i - y_j||^2),  x: [8192, 64], y: [8192, 64].

Strategy: shard rows of x across 8 cores (each computes a [1024, 8192]
tile), replicate y.  The squared distance is produced directly by matmul
via augmented vectors:

    u_i = [-2*x_i, |x_i|^2, 1]   (K = 66)
    v_j = [   y_j,       1, |y_j|^2]

so  u_i . v_j = |x_i|^2 + |y_j|^2 - 2 x_i.y_j = dist2[i, j].  PSUM then
holds dist2 directly and one ScalarE activation computes
exp(-gamma * dist2) per [128, 512] tile — no vector-engine work at all.
Output is staged into [128, 8192] SBUF strips so each store is a single
4 MB DMA (near peak HBM write bandwidth).

MODE selects the matmul precision strategy:
  "f32"    — native fp32 matmul (4 cycles/row on the PE).
  "f32r"   — single-pass fp32 (tf32-like, 1 cycle/row), reduced precision.
  "bf16x3" — split each operand a = hi + lo in bf16 and accumulate
             hi*hi + hi*lo + lo*hi in PSUM (3 bf16 matmuls, ~18-bit
             effective mantissa, 3 cycles/row).
"""

import numpy as np

N_X, N_Y, D = 8192, 8192, 64
N_CORES = 8
N_PER = N_X // N_CORES  # rows of x per core
K_AUG = D + 3  # 67: [-2x, x2-2D, 1, 1] . [y, 1, y2-2D, 2D]

MODE = "f32r"

# Filled by kernel() with the BassKernelResults of the last run
# (test.py reads exec_time_ns from here when BASS_TRACE=1).
LAST_RESULTS = None

_BUILD_CACHE = {}


def _build(gamma: float, n_per: int, m_tot: int, mode: str):
    """Build + compile the single-core Bass program (same on all cores)."""
    import concourse.bacc as bacc
    import concourse.mybir as mybir
    import concourse.tile as tile

    key = (gamma, n_per, m_tot, mode)
    if key in _BUILD_CACHE:
        return _BUILD_CACHE[key]

    dt = mybir.dt
    in_dt = {"f32": dt.float32, "f32r": dt.float32r, "bf16x3": dt.bfloat16}[mode]
    nsplit = 2 if mode == "bf16x3" else 1

    nc = bacc.Bacc("TRN2", target_bir_lowering=False, debug=False)
    ut_d = [
        nc.dram_tensor(f"ut{i}", [K_AUG, n_per], in_dt, kind="ExternalInput").ap()
        for i in range(nsplit)
    ]
    vt_d = [
        nc.dram_tensor(f"vt{i}", [K_AUG, m_tot], in_dt, kind="ExternalInput").ap()
        for i in range(nsplit)
    ]
    out_d = nc.dram_tensor("out", [n_per, m_tot], dt.float32, kind="ExternalOutput").ap()

    MB = n_per // 128  # M-blocks (output partition tiles)
    CHUNK = 2048  # ACT granularity: 4 PSUM banks per activation op
    NCHUNK = m_tot // CHUNK
    JB = CHUNK // 512  # matmuls (PSUM banks) per chunk

    OUT_CHUNK = 4096  # output DMA granularity (2 MB per transfer)

    with tile.TileContext(nc) as tc:
        with (
            tc.tile_pool(name="const", bufs=1) as cpool,
            tc.tile_pool(name="psum", bufs=2, space="PSUM") as psum_pool,
            tc.tile_pool(name="strip", bufs=3) as strip_pool,
        ):
            ut_s = []
            vt_s = []
            for i in range(nsplit):
                u = cpool.tile([K_AUG, n_per], in_dt, tag=f"ut{i}")
                nc.sync.dma_start(u[:], ut_d[i][:])
                ut_s.append(u)
            for i in range(nsplit):
                # chunked load so the first matmuls start after ~0.5 MB
                v = cpool.tile([K_AUG, m_tot], in_dt, tag=f"vt{i}")
                for c in range(NCHUNK):
                    csl = slice(c * CHUNK, (c + 1) * CHUNK)
                    nc.sync.dma_start(v[:, csl], vt_d[i][:, csl])
                vt_s.append(v)

            for m in range(MB):
                strip = strip_pool.tile([128, m_tot], dt.float32)
                msl = slice(m * 128, (m + 1) * 128)
                for c in range(NCHUNK):
                    csl = slice(c * CHUNK, (c + 1) * CHUNK)
                    ps = psum_pool.tile([128, CHUNK], dt.float32)
                    # one matmul (or one split-accumulation group) per PSUM bank
                    if nsplit == 1:
                        for j in range(JB):
                            jsl = slice(j * 512, (j + 1) * 512)
                            vsl = slice(c * CHUNK + j * 512, c * CHUNK + (j + 1) * 512)
                            nc.tensor.matmul(
                                ps[:, jsl], ut_s[0][:, msl], vt_s[0][:, vsl]
                            )
                    else:
                        # hi*hi, hi*lo (same weights), then lo*hi — grouped by
                        # weights so the stationary operand reloads rarely
                        for uu, vv, st, sp in (
                            (0, 0, True, False),
                            (0, 1, False, False),
                            (1, 0, False, True),
                        ):
                            for j in range(JB):
                                jsl = slice(j * 512, (j + 1) * 512)
                                vsl = slice(
                                    c * CHUNK + j * 512, c * CHUNK + (j + 1) * 512
                                )
                                nc.tensor.matmul(
                                    ps[:, jsl],
                                    ut_s[uu][:, msl],
                                    vt_s[vv][:, vsl],
                                    start=st,
                                    stop=sp,
                                )
                    nc.scalar.activation(
                        strip[:, csl],
                        ps[:],
                        mybir.ActivationFunctionType.Exp,
                        scale=-gamma,
                    )
                for oc in range(m_tot // OUT_CHUNK):
                    osl = slice(oc * OUT_CHUNK, (oc + 1) * OUT_CHUNK)
                    nc.sync.dma_start(out_d[msl, osl], strip[:, osl])

    nc.compile()
    _BUILD_CACHE[key] = nc
    return nc


def _augment(x: np.ndarray, y: np.ndarray):
    """Host-side prep: build transposed augmented operands (O(N*D) work)."""
    x = np.asarray(x, dtype=np.float32)
    y = np.asarray(y, dtype=np.float32)
    x2 = np.einsum("nd,nd->n", x, x).astype(np.float32)
    y2 = np.einsum("nd,nd->n", y, y).astype(np.float32)

    # Center the squared norms around their mean (E|x|^2 = D for unit-normal
    # data): the matmul then produces dist2 - 2D with small-magnitude
    # operands (better for the reduced-precision f32r path), and exp()'s
    # bias adds the -gamma*2D shift back.
    ut = np.empty((K_AUG, x.shape[0]), dtype=np.float32)
    ut[:D] = (-2.0 * x).T
    ut[D] = x2 - float(D)
    ut[D + 1] = 1.0
    ut[D + 2] = 1.0

    vt = np.empty((K_AUG, y.shape[0]), dtype=np.float32)
    vt[:D] = y.T
    vt[D] = 1.0
    vt[D + 1] = y2 - float(D)
    vt[D + 2] = 2.0 * float(D)
    return ut, vt


def _split_bf16(a32: np.ndarray):
    import ml_dtypes

    hi = a32.astype(ml_dtypes.bfloat16)
    lo = (a32 - hi.astype(np.float32)).astype(ml_dtypes.bfloat16)
    return [hi, lo]


def kernel(x: np.ndarray, y: np.ndarray, gamma: np.ndarray) -> np.ndarray:
    global LAST_RESULTS
    from concourse.bass_utils import run_bass_kernel_spmd

    gamma_f = float(np.asarray(gamma).reshape(()))
    ut, vt = _augment(x, y)

    nc = _build(gamma_f, N_PER, N_Y, MODE)

    if MODE == "bf16x3":
        uts = _split_bf16(ut)
        vts = _split_bf16(vt)
    else:
        uts, vts = [ut], [vt]

    in_maps = []
    for c in range(N_CORES):
        m = {}
        for i, u in enumerate(uts):
            m[f"ut{i}"] = np.ascontiguousarray(u[:, c * N_PER : (c + 1) * N_PER])
        for i, v in enumerate(vts):
            m[f"vt{i}"] = v
        in_maps.append(m)

    res = run_bass_kernel_spmd(nc, in_maps, core_ids=list(range(N_CORES)))
    LAST_RESULTS = res
    return np.concatenate([res.results[c]["out"] for c in range(N_CORES)], axis=0)



# revision 2
# speedup vs baseline: 2.3922x; 2.3922x over previous
"""RBF (Gaussian) kernel matrix on 8 TRN2 NeuronCores.

out[i, j] = exp(-gamma * ||x_i - y_j||^2),  x: [8192, 64], y: [8192, 64].

Strategy: shard rows of x across 8 cores (each computes a [1024, 8192]
tile), replicate y.  The squared distance is produced directly by matmul
via augmented vectors:

    u_i = [-2*x_i, |x_i|^2 - D, 1, 1]   (K = 67)
    v_j = [   y_j,           1, |y_j|^2 - D, 2D]

so  u_i . v_j = |x_i|^2 + |y_j|^2 - 2 x_i.y_j = dist2[i, j].  PSUM then
holds dist2 directly and one ScalarE activation computes
exp(-gamma * dist2) per [128, 2048] tile — no vector-engine work at all.

Perf-critical details (from the baseline's perfetto profile):

* Input DRAM tensors are zero-padded from 67 to 128 partitions.  HWDGE
  only spreads a DMA's descriptors across the 16 SDMA engines when the
  SBUF side covers all 128 partitions; a 67-partition load lands on ONE
  engine and serializes the whole kernel behind it (the baseline spent
  120us draining input descriptors through a single ~26 GB/s engine).
  The matmul still uses the [0:67] partition slice, so LDWEIGHTS stays
  67 rows.

* The exp result lies in [0, 1]; storing it as bf16 (rel. roundoff
  2^-9) keeps the absmax-relative error ~0.2% while halving output HBM
  traffic (16.8 MB/core instead of 33.5 MB).  The host upcasts to f32.

* f32r (tf32-like single-pass) matmul streams 1 column/cycle; the
  squared norms are centered around their mean (E|x|^2 = D) so the
  reduced-precision accumulation stays accurate.
"""

import numpy as np

N_X, N_Y, D = 8192, 8192, 64
N_CORES = 8
N_PER = N_X // N_CORES  # rows of x per core
K_AUG = D + 3  # 67: [-2x, x2-D, 1, 1] . [y, 1, y2-D, 2D]
K_PAD = 128  # DMA-side partition padding (descriptor spread)

# Filled by kernel() with the BassKernelResults of the last run
# (test.py reads exec_time_ns from here when BASS_TRACE=1).
LAST_RESULTS = None

_BUILD_CACHE = {}


def _build(gamma: float, n_per: int, m_tot: int):
    """Build + compile the single-core Bass program (same on all cores)."""
    import concourse.bacc as bacc
    import concourse.mybir as mybir
    import concourse.tile as tile

    key = (gamma, n_per, m_tot)
    if key in _BUILD_CACHE:
        return _BUILD_CACHE[key]

    dt = mybir.dt

    nc = bacc.Bacc("TRN2", target_bir_lowering=False, debug=False)
    ut_d = nc.dram_tensor("ut", [K_PAD, n_per], dt.float32r, kind="ExternalInput").ap()
    vt_d = nc.dram_tensor("vt", [K_PAD, m_tot], dt.float32r, kind="ExternalInput").ap()
    out_d = nc.dram_tensor("out", [n_per, m_tot], dt.bfloat16, kind="ExternalOutput").ap()

    MB = n_per // 128  # M-blocks (output partition tiles)
    CHUNK = 2048  # ACT granularity: 4 PSUM banks per activation op
    NCHUNK = m_tot // CHUNK
    JB = CHUNK // 512  # matmuls (PSUM banks) per chunk

    OUT_CHUNK = 4096  # output DMA granularity (1 MB per transfer in bf16)

    with tile.TileContext(nc) as tc:
        with (
            tc.tile_pool(name="const", bufs=1) as cpool,
            tc.tile_pool(name="psum", bufs=2, space="PSUM") as psum_pool,
            tc.tile_pool(name="strip", bufs=3) as strip_pool,
        ):
            ut_s = cpool.tile([K_PAD, n_per], dt.float32r, tag="ut")
            nc.sync.dma_start(ut_s[:], ut_d[:])
            # chunked load so the first matmuls start after ~1 MB
            vt_s = cpool.tile([K_PAD, m_tot], dt.float32r, tag="vt")
            for c in range(NCHUNK):
                csl = slice(c * CHUNK, (c + 1) * CHUNK)
                nc.sync.dma_start(vt_s[:, csl], vt_d[:, csl])

            for m in range(MB):
                strip = strip_pool.tile([128, m_tot], dt.bfloat16)
                msl = slice(m * 128, (m + 1) * 128)
                for c in range(NCHUNK):
                    csl = slice(c * CHUNK, (c + 1) * CHUNK)
                    ps = psum_pool.tile([128, CHUNK], dt.float32)
                    for j in range(JB):
                        jsl = slice(j * 512, (j + 1) * 512)
                        vsl = slice(c * CHUNK + j * 512, c * CHUNK + (j + 1) * 512)
                        nc.tensor.matmul(
                            ps[:, jsl], ut_s[:K_AUG, msl], vt_s[:K_AUG, vsl]
                        )
                    nc.scalar.activation(
                        strip[:, csl],
                        ps[:],
                        mybir.ActivationFunctionType.Exp,
                        scale=-gamma,
                    )
                for oc in range(m_tot // OUT_CHUNK):
                    osl = slice(oc * OUT_CHUNK, (oc + 1) * OUT_CHUNK)
                    nc.sync.dma_start(out_d[msl, osl], strip[:, osl])

    nc.compile()
    _BUILD_CACHE[key] = nc
    return nc


def _augment(x: np.ndarray, y: np.ndarray):
    """Host-side prep: build transposed augmented operands (O(N*D) work).

    Rows K_AUG..K_PAD-1 are zero padding so the HBM->SBUF DMA covers all
    128 partitions (descriptor spread across the 16 SDMA engines).
    """
    x = np.asarray(x, dtype=np.float32)
    y = np.asarray(y, dtype=np.float32)
    x2 = np.einsum("nd,nd->n", x, x).astype(np.float32)
    y2 = np.einsum("nd,nd->n", y, y).astype(np.float32)

    # Center the squared norms around their mean (E|x|^2 = D for unit-normal
    # data): the matmul addends then have small magnitudes, which keeps the
    # reduced-precision f32r accumulation accurate.
    ut = np.zeros((K_PAD, x.shape[0]), dtype=np.float32)
    ut[:D] = (-2.0 * x).T
    ut[D] = x2 - float(D)
    ut[D + 1] = 1.0
    ut[D + 2] = 1.0

    vt = np.zeros((K_PAD, y.shape[0]), dtype=np.float32)
    vt[:D] = y.T
    vt[D] = 1.0
    vt[D + 1] = y2 - float(D)
    vt[D + 2] = 2.0 * float(D)
    return ut, vt


def kernel(x: np.ndarray, y: np.ndarray, gamma: np.ndarray) -> np.ndarray:
    global LAST_RESULTS
    from concourse.bass_utils import run_bass_kernel_spmd

    gamma_f = float(np.asarray(gamma).reshape(()))
    ut, vt = _augment(x, y)

    nc = _build(gamma_f, N_PER, N_Y)

    in_maps = []
    for c in range(N_CORES):
        in_maps.append(
            {
                "ut": np.ascontiguousarray(ut[:, c * N_PER : (c + 1) * N_PER]),
                "vt": vt,
            }
        )

    res = run_bass_kernel_spmd(nc, in_maps, core_ids=list(range(N_CORES)))
    LAST_RESULTS = res
    out = np.concatenate(
        [np.asarray(res.results[c]["out"]) for c in range(N_CORES)], axis=0
    )
    return out.astype(np.float32)


# revision 3
# speedup vs baseline: 2.4652x; 1.0305x over previous
"""RBF (Gaussian) kernel matrix on 8 TRN2 NeuronCores.

out[i, j] = exp(-gamma * ||x_i - y_j||^2),  x: [8192, 64], y: [8192, 64].

Strategy: shard rows of x across 8 cores (each computes a [1024, 8192]
tile), replicate y.  The squared distance is produced directly by matmul
via augmented vectors:

    u_i = [-2*x_i, |x_i|^2 - D, 1, 1]   (K = 67)
    v_j = [   y_j,           1, |y_j|^2 - D, 2D]

so  u_i . v_j = |x_i|^2 + |y_j|^2 - 2 x_i.y_j = dist2[i, j].  PSUM then
holds dist2 directly and one ScalarE activation computes
exp(-gamma * dist2) per [128, 2048] tile — no vector-engine work at all.

Perf-critical details (from the baseline's perfetto profile):

* Input DRAM tensors are zero-padded from 67 to 128 partitions.  HWDGE
  only spreads a DMA's descriptors across the 16 SDMA engines when the
  SBUF side covers all 128 partitions; a 67-partition load lands on ONE
  engine and serializes the whole kernel behind it (the baseline spent
  120us draining input descriptors through a single ~26 GB/s engine).
  The matmul still uses the [0:67] partition slice, so LDWEIGHTS stays
  67 rows.

* The exp result lies in [0, 1]; storing it as bf16 (rel. roundoff
  2^-9) keeps the absmax-relative error ~0.2% while halving output HBM
  traffic (16.8 MB/core instead of 33.5 MB).  The host upcasts to f32.

* f32r (tf32-like single-pass) matmul streams 1 column/cycle; the
  squared norms are centered around their mean (E|x|^2 = D) so the
  reduced-precision accumulation stays accurate.
"""

import numpy as np

N_X, N_Y, D = 8192, 8192, 64
N_CORES = 8
N_PER = N_X // N_CORES  # rows of x per core
K_AUG = D + 3  # 67: [-2x, x2-D, 1, 1] . [y, 1, y2-D, 2D]
K_PAD = 128  # DMA-side partition padding (descriptor spread)

# Filled by kernel() with the BassKernelResults of the last run
# (test.py reads exec_time_ns from here when BASS_TRACE=1).
LAST_RESULTS = None

_BUILD_CACHE = {}


def _build(gamma: float, n_per: int, m_tot: int):
    """Build + compile the single-core Bass program (same on all cores)."""
    import concourse.bacc as bacc
    import concourse.mybir as mybir
    import concourse.tile as tile

    key = (gamma, n_per, m_tot)
    if key in _BUILD_CACHE:
        return _BUILD_CACHE[key]

    dt = mybir.dt

    nc = bacc.Bacc("TRN2", target_bir_lowering=False, debug=False)
    ut_d = nc.dram_tensor("ut", [K_PAD, n_per], dt.float32r, kind="ExternalInput").ap()
    vt_d = nc.dram_tensor("vt", [K_PAD, m_tot], dt.float32r, kind="ExternalInput").ap()
    out_d = nc.dram_tensor("out", [n_per, m_tot], dt.bfloat16, kind="ExternalOutput").ap()

    MB = n_per // 128  # M-blocks (output partition tiles)
    CHUNK = 2048  # ACT granularity: 4 PSUM banks per activation op

    # Per-strip chunk schedules.  Strip 0 warms up with small chunks so the
    # first ACTIVATE fires as soon as the first 512 vt columns land; the
    # last strip tapers off so the final activation + store are small
    # (shorter drain tail).  Middle strips run at the steady 2048 rhythm.
    warmup = [512, 512, 1024] + [CHUNK] * 3
    steady = [CHUNK] * (m_tot // CHUNK)
    taper = [CHUNK] * 3 + [1024, 512, 512]
    schedules = [warmup] + [steady] * (MB - 2) + [taper]

    # Output DMA boundaries: 4096-col (1 MB) pieces in steady state; the
    # last strip stores per-chunk so the tail transfer is tiny.
    def out_pieces(sched):
        if sched is taper:
            bounds, acc = [0], 0
            for c in sched:
                acc += c
                bounds.append(acc)
            return bounds
        return list(range(0, m_tot + 1, 4096))

    with tile.TileContext(nc) as tc:
        with (
            tc.tile_pool(name="const", bufs=1) as cpool,
            tc.tile_pool(name="psum", bufs=2, space="PSUM") as psum_pool,
            tc.tile_pool(name="strip", bufs=3) as strip_pool,
        ):
            ut_s = cpool.tile([K_PAD, n_per], dt.float32r, tag="ut")
            nc.sync.dma_start(ut_s[:], ut_d[:])
            # load vt in pieces matching strip 0's warmup schedule so the
            # first matmuls start after ~0.25 MB
            vt_s = cpool.tile([K_PAD, m_tot], dt.float32r, tag="vt")
            off = 0
            for c in warmup:
                nc.sync.dma_start(vt_s[:, off : off + c], vt_d[:, off : off + c])
                off += c

            for m in range(MB):
                sched = schedules[m]
                strip = strip_pool.tile([128, m_tot], dt.bfloat16)
                msl = slice(m * 128, (m + 1) * 128)
                off = 0
                for clen in sched:
                    csl = slice(off, off + clen)
                    ps = psum_pool.tile([128, CHUNK], dt.float32)
                    for j in range(clen // 512):
                        jsl = slice(j * 512, (j + 1) * 512)
                        vsl = slice(off + j * 512, off + (j + 1) * 512)
                        nc.tensor.matmul(
                            ps[:, jsl], ut_s[:K_AUG, msl], vt_s[:K_AUG, vsl]
                        )
                    nc.scalar.activation(
                        strip[:, csl],
                        ps[:, :clen],
                        mybir.ActivationFunctionType.Exp,
                        scale=-gamma,
                    )
                    off += clen
                bounds = out_pieces(sched)
                for lo, hi in zip(bounds[:-1], bounds[1:]):
                    nc.sync.dma_start(out_d[msl, lo:hi], strip[:, lo:hi])

    nc.compile()
    _BUILD_CACHE[key] = nc
    return nc


def _augment(x: np.ndarray, y: np.ndarray):
    """Host-side prep: build transposed augmented operands (O(N*D) work).

    Rows K_AUG..K_PAD-1 are zero padding so the HBM->SBUF DMA covers all
    128 partitions (descriptor spread across the 16 SDMA engines).
    """
    x = np.asarray(x, dtype=np.float32)
    y = np.asarray(y, dtype=np.float32)
    x2 = np.einsum("nd,nd->n", x, x).astype(np.float32)
    y2 = np.einsum("nd,nd->n", y, y).astype(np.float32)

    # Center the squared norms around their mean (E|x|^2 = D for unit-normal
    # data): the matmul addends then have small magnitudes, which keeps the
    # reduced-precision f32r accumulation accurate.
    ut = np.zeros((K_PAD, x.shape[0]), dtype=np.float32)
    ut[:D] = (-2.0 * x).T
    ut[D] = x2 - float(D)
    ut[D + 1] = 1.0
    ut[D + 2] = 1.0

    vt = np.zeros((K_PAD, y.shape[0]), dtype=np.float32)
    vt[:D] = y.T
    vt[D] = 1.0
    vt[D + 1] = y2 - float(D)
    vt[D + 2] = 2.0 * float(D)
    return ut, vt


def kernel(x: np.ndarray, y: np.ndarray, gamma: np.ndarray) -> np.ndarray:
    global LAST_RESULTS
    from concourse.bass_utils import run_bass_kernel_spmd

    gamma_f = float(np.asarray(gamma).reshape(()))
    ut, vt = _augment(x, y)

    nc = _build(gamma_f, N_PER, N_Y)

    in_maps = []
    for c in range(N_CORES):
        in_maps.append(
            {
                "ut": np.ascontiguousarray(ut[:, c * N_PER : (c + 1) * N_PER]),
                "vt": vt,
            }
        )

    res = run_bass_kernel_spmd(nc, in_maps, core_ids=list(range(N_CORES)))
    LAST_RESULTS = res
    out = np.concatenate(
        [np.asarray(res.results[c]["out"]) for c in range(N_CORES)], axis=0
    )
    return out.astype(np.float32)
